# revision 18
# baseline (speedup 1.0000x reference)
"""Trainium2 Bass kernel for AutoregressiveMultimodalRNN.

Math (reference):
  LSTM(256 steps, B=8, IN=256, H=128) -> hs [64, 4096]
  q,k,v = hs @ W{q,k,v}.T + b        (4096x4096 each)
  r = softmax(q*k, -1) * v           (elementwise)
  4 stacked linears (4096x4096) then Wout (1x4096), sigmoid.

Key algebraic transform (host, float64, exact):
  The 4 linears + Wout compose into a single vector w_eff[4096] and scalar
  c_eff:  out = sigmoid( (r @ w_eff) + c_eff ).
  w_eff folds into Wv (per-row scale), so on device:
  out_row = sigmoid( sum_j exp(t_j) v'_j / sum_j exp(t_j) + c_eff ),
  t = q*k elementwise.  Device weight traffic: only Wq,Wk,Wv' (192MB),
  column-sharded 8 ways -> 24MB/core.

Sharding: Wq/Wk/Wv column-parallel (each core owns 512 of 4096 features);
LSTM replicated on every core; per-core partial (sum_exp, dot) [64,2]
reduced on host (512B/core), then sigmoid.
"""

import sys, os

sys.path.insert(0, "/opt/trn_rl_repo")

import numpy as np

NCH, S, B, IN, H = 8, 32, 8, 256, 128
D = S * H            # 4096
NT = NCH * S         # 256 lstm steps
R = NCH * B          # 64 rows of hs
NCORES = 8
DM = D // NCORES     # 512 features per core
NWBUF = 56           # weight-tile prefetch slots (x 256KB)

# packed consts layout (columns of a [128, CONST_COLS] f32 block)
C_XT = 0                       # XT, kt-major: kt*2048 + t*8+b   (2*2048)
C_WIH = C_XT + 2 * NT * B      # WihT kt-major (2*512)
C_WHH = C_WIH + 2 * 4 * H      # WhhT (512)
C_B4 = C_WHH + 4 * H           # per-gate bias (4)
C_Z = C_B4 + 4                 # zeros for h0,c0 (16)
C_ROW0 = C_Z + 2 * B           # row0-only data: ones(64) | bq,bk,bv (1536)
CONST_COLS = C_ROW0 + R + 3 * (D // NCORES)

_CACHE = {}


def _build_nc():
    import concourse.bass as bass
    import concourse.mybir as mybir
    from concourse import tile

    f32 = mybir.dt.float32
    AF = mybir.ActivationFunctionType
    OP = mybir.AluOpType

    nc = bass.Bass()

    consts = nc.declare_dram_parameter("consts", [128, CONST_COLS], f32, isOutput=False)
    wq = nc.declare_dram_parameter("wq", [D, DM], f32, isOutput=False)
    wk = nc.declare_dram_parameter("wk", [D, DM], f32, isOutput=False)
    wv = nc.declare_dram_parameter("wv", [D, DM], f32, isOutput=False)
    out = nc.declare_dram_parameter("out", [R, 2], f32, isOutput=True)

    with tile.TileContext(nc) as tc:
        with (
            tc.tile_pool(name="const", bufs=1) as cpool,
            tc.tile_pool(name="warena", bufs=1) as wpool_a,
        ):
            consts_t = cpool.tile([128, CONST_COLS], f32)
            garena = wpool_a.tile([128, 4 * NT * B], f32)   # [hid, gate*t*b] 4MB
            harena = wpool_a.tile([128, NT * B], f32)       # hs^T arena, col=s*64+c*8+b

            nc.sync.dma_start(consts_t[:], consts[:])
            xt_t = consts_t[:, C_XT:C_XT + 2 * NT * B]
            wih_t = consts_t[:, C_WIH:C_WIH + 2 * 4 * H]
            whh_t = consts_t[:, C_WHH:C_WHH + 4 * H]
            b4_t = consts_t[:, C_B4:C_B4 + 4]
            h0 = consts_t[:, C_Z:C_Z + B]
            c0 = consts_t[:, C_Z + B:C_Z + 2 * B]
            ones_t = consts_t[0:1, C_ROW0:C_ROW0 + R]
            bqk_t = consts_t[0:1, C_ROW0 + R:C_ROW0 + R + 3 * DM]

            PH = os.environ.get("KPH", "123e")
            # ---- Phase 1: G_ih = Wih_r @ X  (gates-on-partitions), + gate bias
            with tc.tile_pool(name="psum1", bufs=4, space="PSUM") as psum1:
                for g in range(4 if "1" in PH else 0):
                    for nt_i in range(4):  # 4 x 512 cols of 2048
                        ps = psum1.tile([128, 512], f32, tag="ps1")
                        for kt in range(2):
                            nc.tensor.matmul(
                                ps[:],
                                wih_t[:, kt * 512 + g * 128: kt * 512 + (g + 1) * 128],
                                xt_t[:, kt * 2048 + nt_i * 512: kt * 2048 + (nt_i + 1) * 512],
                                start=(kt == 0),
                                stop=(kt == 1),
                            )
                        nc.vector.tensor_scalar(
                            out=garena[:, g * 2048 + nt_i * 512: g * 2048 + (nt_i + 1) * 512],
                            in0=ps[:],
                            scalar1=b4_t[:, g:g + 1],
                            scalar2=None,
                            op0=OP.add,
                        )

            # ---- Phase 2: LSTM recurrence, layout [hid(128), ...] on partitions
            gv = garena[:].rearrange("p (g t b) -> p g t b", g=4, t=NT, b=B)
            with (
                tc.tile_pool(name="psum2", bufs=4, space="PSUM") as psum2,
                tc.tile_pool(name="lstm", bufs=4) as lpool,
                tc.tile_pool(name="cstate", bufs=2) as cs_pool,
            ):
                c_prev = c0
                for t in range(NT if "2" in PH else 0):
                    chunk, s = divmod(t, S)
                    if t == 0:
                        h_prev = h0
                    else:
                        pc, ps_ = divmod(t - 1, S)
                        h_prev = harena[:, ps_ * 64 + pc * 8: ps_ * 64 + pc * 8 + 8]
                    pt = psum2.tile([128, 4 * B], f32, tag="pt")
                    for g in range(4):
                        nc.tensor.matmul(
                            pt[:, g * B:(g + 1) * B],
                            whh_t[:, g * 128:(g + 1) * 128],
                            h_prev,
                            start=True,
                            stop=True,
                        )
                    g_sb = lpool.tile([128, 4 * B], f32, tag="g_sb")
                    nc.vector.tensor_tensor(
                        out=g_sb[:].rearrange("p (g b) -> p g b", g=4),
                        in0=pt[:].rearrange("p (g b) -> p g b", g=4),
                        in1=gv[:, :, t, :],
                        op=OP.add,
                    )
                    s_sb = lpool.tile([128, 4 * B], f32, tag="s_sb")
                    # gate order i,f,o,g: sigmoid on first 24, tanh on last 8
                    nc.scalar.activation(s_sb[:, 0:3 * B], g_sb[:, 0:3 * B], AF.Sigmoid)
                    nc.scalar.activation(s_sb[:, 3 * B:4 * B], g_sb[:, 3 * B:4 * B], AF.Tanh)
                    t1 = lpool.tile([128, B], f32, tag="t1")
                    t2 = lpool.tile([128, B], f32, tag="t2")
                    nc.vector.tensor_tensor(out=t1[:], in0=s_sb[:, B:2 * B], in1=(c_prev if t == 0 else c_prev[:]), op=OP.mult)
                    nc.vector.tensor_tensor(out=t2[:], in0=s_sb[:, 0:B], in1=s_sb[:, 3 * B:4 * B], op=OP.mult)
                    c_new = cs_pool.tile([128, B], f32, tag="c")
                    nc.vector.tensor_tensor(out=c_new[:], in0=t1[:], in1=t2[:], op=OP.add)
                    tc_t = lpool.tile([128, B], f32, tag="tc")
                    nc.scalar.activation(tc_t[:], c_new[:], AF.Tanh)
                    hcol = s * 64 + chunk * 8
                    nc.vector.tensor_tensor(
                        out=harena[:, hcol:hcol + 8],
                        in0=s_sb[:, 2 * B:3 * B],
                        in1=tc_t[:],
                        op=OP.mult,
                    )
                    c_prev = c_new

            # ---- Phase 3: q,k,v = hs @ W.T + b  (column shard, 512 wide)
            with (
                tc.tile_pool(name="psum3", bufs=1, space="PSUM") as psum3,
                tc.tile_pool(name="wtiles", bufs=NWBUF) as wt_pool,
                tc.tile_pool(name="epi", bufs=1) as epool,
            ):
                psq = psum3.tile([R, DM], f32, tag="psq")
                psk = psum3.tile([R, DM], f32, tag="psk")
                psv = psum3.tile([R, DM], f32, tag="psv")
                bias_on = os.environ.get("KBIAS", "1") == "1"
                qkv_list = ((wq, psq), (wk, psk), (wv, psv)) if "3" in PH else ()
                for wi, (wdram, pst) in enumerate(qkv_list):
                    for s in range(32):
                        wt = wt_pool.tile([128, DM], f32, tag="w")
                        nc.sync.dma_start(wt[:], wdram[s * 128:(s + 1) * 128, :])
                        nc.tensor.matmul(
                            pst[:],
                            harena[:, s * 64:(s + 1) * 64],
                            wt[:],
                            start=(s == 0),
                            stop=(s == 31 and not bias_on),
                        )
                    if os.environ.get("KBIAS", "1") == "1":
                        nc.tensor.matmul(
                            pst[:], ones_t, bqk_t[:, wi * DM:(wi + 1) * DM],
                            start=False, stop=True,
                        )

                # ---- Phase 4: t=q*k; e=exp(t); partials (sum e, sum e*v)
                o_sb = epool.tile([R, 2], f32)
                if "e" in PH and "3" in PH:
                    k_sb = epool.tile([R, DM], f32)
                    nc.scalar.copy(k_sb[:], psk[:])
                    t_sb = epool.tile([R, DM], f32)
                    nc.vector.tensor_tensor(out=t_sb[:], in0=psq[:], in1=k_sb[:], op=OP.mult)
                    e_sb = epool.tile([R, DM], f32)
                    s_part = epool.tile([R, 1], f32)
                    nc.scalar.activation(e_sb[:], t_sb[:], AF.Exp, accum_out=s_part[:])
                    u_sb = epool.tile([R, DM], f32)
                    nc.vector.tensor_tensor(out=u_sb[:], in0=e_sb[:], in1=psv[:], op=OP.mult)
                    p_part = epool.tile([R, 1], f32)
                    nc.vector.tensor_reduce(
                        out=p_part[:], in_=u_sb[:], axis=mybir.AxisListType.X, op=OP.add
                    )
                    nc.vector.tensor_copy(o_sb[:, 0:1], s_part[:])
                    nc.vector.tensor_copy(o_sb[:, 1:2], p_part[:])
                else:
                    nc.gpsimd.memset(o_sb[:], 1.0)
                nc.gpsimd.dma_start(out[:], o_sb[:])

    _split_multi_waits(nc)
    return nc


def _split_multi_waits(nc):
    """This walrus build lowers at most one on_wait per instruction; hoist
    extras into standalone EventSemaphore waits on the same engine."""
    import concourse.mybir as mybir

    for bb in nc.main_func.blocks:
        insts = list(bb.instructions)
        changed, out = False, []
        for ins in insts:
            si = ins.sync_info
            if si is not None and si.on_wait is not None and len(si.on_wait) > 1:
                waits = list(si.on_wait)
                for idx, w in enumerate(waits[:-1]):
                    ev = mybir.InstEventSemaphore(name=f"wsplit_{ins.name}_{idx}")
                    ev.engine = ins.engine
                    ev.sync_info = mybir.SyncInfo(on_wait=[w], on_update=[])
                    out.append(ev)
                ins.sync_info = mybir.SyncInfo(
                    on_wait=[waits[-1]], on_update=list(si.on_update or [])
                )
                changed = True
            out.append(ins)
        if changed:
            bb.instructions = out


def _prep_host(inputs):
    x = np.asarray(inputs["x"], np.float32)
    Wih = np.asarray(inputs["Wih"], np.float32)
    Whh = np.asarray(inputs["Whh"], np.float32)
    bih = np.asarray(inputs["bih"], np.float32)
    bhh = np.asarray(inputs["bhh"], np.float32)
    Wq = np.asarray(inputs["Wq"], np.float32)
    bq = np.asarray(inputs["bq"], np.float32)
    Wk = np.asarray(inputs["Wk"], np.float32)
    bk = np.asarray(inputs["bk"], np.float32)
    Wv = np.asarray(inputs["Wv"], np.float32)
    bv = np.asarray(inputs["bv"], np.float32)
    Wl = np.asarray(inputs["Wl"], np.float64)
    bl = np.asarray(inputs["bl"], np.float64)
    Wout = np.asarray(inputs["Wout"], np.float64)
    bout = np.asarray(inputs["bout"], np.float64)

    # fold linear stack + Wout -> w_eff [D], c_eff scalar (exact algebra, f64)
    v = Wout.copy()            # [1, D]
    c = bout.copy()            # [1]
    for i in (3, 2, 1, 0):
        c = c + v @ bl[i]
        v = v @ Wl[i]
    w_eff = v[0]               # [D]
    c_eff = float(c[0])

    Wv_p = (Wv.astype(np.float64) * w_eff[:, None]).astype(np.float32)
    bv_p = (bv.astype(np.float64) * w_eff).astype(np.float32)

    # gate reorder (i,f,g,o) -> (i,f,o,g)
    idx = np.concatenate(
        [np.arange(0, H), np.arange(H, 2 * H), np.arange(3 * H, 4 * H), np.arange(2 * H, 3 * H)]
    )
    Wih_r, Whh_r, b_r = Wih[idx], Whh[idx], (bih + bhh)[idx]

    xt2 = x.reshape(NT * B, IN).T                    # [256, 2048]
    wihT2 = Wih_r.T                                  # [256, 512]
    whhT = Whh_r.T                                   # [128, 512]
    b4 = b_r.reshape(4, H).T                         # [128, 4]

    in_maps = []
    for m in range(NCORES):
        sl = slice(m * DM, (m + 1) * DM)
        consts = np.zeros((128, CONST_COLS), np.float32)
        for kt in range(2):
            consts[:, C_XT + kt * NT * B: C_XT + (kt + 1) * NT * B] = \
                xt2[kt * 128:(kt + 1) * 128]
            consts[:, C_WIH + kt * 4 * H: C_WIH + (kt + 1) * 4 * H] = \
                wihT2[kt * 128:(kt + 1) * 128]
        consts[:, C_WHH:C_WHH + 4 * H] = whhT
        consts[:, C_B4:C_B4 + 4] = b4
        consts[0, C_ROW0:C_ROW0 + R] = 1.0
        consts[0, C_ROW0 + R:] = np.concatenate([bq[sl], bk[sl], bv_p[sl]])
        in_maps.append(
            dict(
                consts=consts,
                wq=np.ascontiguousarray(Wq[sl].T),
                wk=np.ascontiguousarray(Wk[sl].T),
                wv=np.ascontiguousarray(Wv_p[sl].T),
            )
        )
    return in_maps, c_eff


def _ensure_ntff_hook():
    """antenv.axon_hooks is missing in this image; provide a shim backed by
    ctypes calls into libaxon_pjrt.so (mirrors trn_boot.py)."""
    try:
        from antenv.axon_hooks import get_axon_ntff_profile_hook  # noqa: F401
        return
    except ImportError:
        pass
    import types, ctypes, contextlib

    so_path = "/opt/axon/libaxon_pjrt.so"
    lib = ctypes.CDLL(so_path)
    if not hasattr(lib, "axon_start_nrt_profile"):
        return
    lib.axon_start_nrt_profile.argtypes = [
        ctypes.POINTER(ctypes.c_int64), ctypes.c_size_t,
    ]
    lib.axon_start_nrt_profile.restype = ctypes.c_int64
    lib.axon_stop_nrt_profile.argtypes = [ctypes.c_char_p]
    lib.axon_stop_nrt_profile.restype = ctypes.c_int64

    @contextlib.contextmanager
    def _hook(output_dir, device_ids):
        import jax
        jax.devices()
        if device_ids:
            ids = (ctypes.c_int64 * len(device_ids))(*device_ids)
            rc = lib.axon_start_nrt_profile(ids, len(device_ids))
        else:
            rc = lib.axon_start_nrt_profile(None, 0)
        if rc != 0:
            raise RuntimeError(f"axon_start_nrt_profile rc={rc}")
        try:
            yield
        finally:
            n = lib.axon_stop_nrt_profile(str(output_dir).encode())
            print(f"profile: {n} file(s) written to {output_dir}", file=sys.stderr)

    mod = types.ModuleType("antenv.axon_hooks")
    _state = {"hook": _hook}
    mod.set_axon_ntff_profile_hook = lambda h: _state.__setitem__("hook", h)
    mod.get_axon_ntff_profile_hook = lambda: _state["hook"]
    sys.modules["antenv.axon_hooks"] = mod
    import antenv
    antenv.axon_hooks = mod


def kernel(**inputs):
    from concourse.bass_utils import run_bass_kernel_spmd

    if "nc" not in _CACHE:
        _CACHE["nc"] = _build_nc()
    nc = _CACHE["nc"]

    in_maps, c_eff = _prep_host(inputs)
    trace = os.environ.get("KTRACE", "0") == "1"
    if trace:
        _ensure_ntff_hook()
        tmpdir = "/tmp/ktrace"
        os.makedirs(tmpdir, exist_ok=True)
    else:
        tmpdir = None
    res = run_bass_kernel_spmd(
        nc, in_maps, core_ids=list(range(NCORES)), trace=trace, tmpdir=tmpdir
    )
    _CACHE["last_exec_ns"] = res.exec_time_ns
    parts = np.stack([np.asarray(res.results[m]["out"]) for m in range(NCORES)])
    S_sum = parts[:, :, 0].sum(axis=0)
    P_sum = parts[:, :, 1].sum(axis=0)
    z = P_sum / S_sum + c_eff
    out = (1.0 / (1.0 + np.exp(-z))).astype(np.float32)
    return out.reshape(NCH, B, 1)
